# revision 14
# baseline (speedup 1.0000x reference)
"""Trainium2 Bass kernel for nn_BigNet: 1000x (Linear(100,100)+ReLU) -> Linear(100,10).

Strategy (data-parallel over 8 NeuronCores, batch 8192 -> 1024/core):
  - Activations live in SBUF transposed: h = [feature(+ones row), batch_cols].
  - Each layer: psum = Waug_l.T @ h_aug  (bias folded in as an extra ones-row
    contraction, K=101 <= 128 so it is free on the PE), then ReLU PSUM->SBUF.
  - ReLU work is split across ScalarE (ACT) and VectorE (DVE) by batch-column
    chunks so neither engine is the serial bottleneck.
  - Weights stream from HBM (40MB can't fit in 24MB SBUF), prefetched in
    groups of layers, double-buffered.
  - Final 100->10 layer folded the same way, output [10, 1024] per core,
    gathered and transposed on the host.
"""

import sys

if "/opt/trn_rl_repo" not in sys.path:
    sys.path.insert(0, "/opt/trn_rl_repo")

import numpy as np

N_LAYERS, D, D_OUT, B, N_CORES = 1000, 100, 10, 8192, 8
K = D + 1  # augmented contraction dim (ones row carries the bias)
B_CORE = B // N_CORES  # 1024 batch columns per core

# Batch-column chunks per layer: first group -> ScalarE relu, second -> VectorE.
# Sizes balance measured op costs: ACT=(FD+312)/1.2GHz, DVE=(FD+152)/0.96GHz.
CHUNKS = (252, 252, 260, 260)
N_ACT_CHUNKS = 2  # chunks 0..1 on ScalarE, rest on VectorE
W_PER_DMA = 8  # layers per weight-prefetch DMA

MM_DTYPE = "float32r"  # "float32" (4 cyc/row, exact) or "float32r" (1 cyc/row)

_BUILT = {}


def _build(mm_dtype):
    import concourse.bacc as bacc
    import concourse.mybir as mybir
    from concourse.tile import TileContext

    f32 = mybir.dt.float32
    # float32r: PE streams at 1 cyc/row (vs 4 for f32); producers (DMA/ACT/DVE)
    # must declare f32r output so walrus sees rounded inputs to the matmul.
    DT = mybir.dt.float32r if mm_dtype == "float32r" else f32

    offs = [0]
    for cw in CHUNKS:
        offs.append(offs[-1] + cw)
    assert offs[-1] == B_CORE

    nc = bacc.Bacc(None, target_bir_lowering=False)
    n_groups = N_LAYERS // W_PER_DMA
    # One contiguous [K, W_PER_DMA*D] block per group so each weight DMA is a
    # single linear HBM burst (strided layout measured ~25GB/s; linear is fast).
    wt_e = nc.dram_tensor(
        "wt", [n_groups, K, D * W_PER_DMA], DT, kind="ExternalInput"
    )
    xt_e = nc.dram_tensor("xt", [K, B_CORE], DT, kind="ExternalInput")
    wft_e = nc.dram_tensor("wft", [K, D_OUT], DT, kind="ExternalInput")
    out_e = nc.dram_tensor("out", [D_OUT, B_CORE], f32, kind="ExternalOutput")

    with TileContext(nc) as tc:
        with (
            tc.tile_pool(name="h", bufs=1) as hpool,
            tc.tile_pool(name="w", bufs=4) as wpool,
            tc.tile_pool(name="ps", bufs=1, space="PSUM") as pspool,
            tc.tile_pool(name="misc", bufs=1) as mpool,
        ):
            wf_tile = mpool.tile([K, D_OUT], DT, tag="wf")
            nc.sync.dma_start(wf_tile[:], wft_e[:])

            # Persistent ping-pong activation tiles, one pair per chunk.
            # Row D (index 100) holds the constant 1.0 that multiplies the
            # bias row of the augmented weights.
            h = [
                [
                    hpool.tile([K, CHUNKS[c]], DT, tag=f"h{p}_{c}", name=f"h{p}_{c}")
                    for c in range(len(CHUNKS))
                ]
                for p in range(2)
            ]
            for c in range(len(CHUNKS)):
                nc.sync.dma_start(h[0][c][:], xt_e[:, offs[c] : offs[c + 1]])
                # Ones row for the odd-parity tiles comes from xt's row 100
                # (memset can't encode float32r).
                nc.sync.dma_start(
                    h[1][c][D:K, :], xt_e[D:K, offs[c] : offs[c + 1]]
                )

            wtile = None
            for l in range(N_LAYERS):
                j = l % W_PER_DMA
                if j == 0:
                    g = l // W_PER_DMA
                    wtile = wpool.tile([K, D * W_PER_DMA], DT, tag="w", name="wtile")
                    eng = nc.sync if g % 2 == 0 else nc.gpsimd
                    eng.dma_start(wtile[:], wt_e[g])
                p, q = l % 2, (l + 1) % 2
                for c, cw in enumerate(CHUNKS):
                    ps = pspool.tile([D, cw], f32, tag=f"ps{c}", name=f"ps{c}")
                    nc.tensor.matmul(ps[:], wtile[:, j * D : (j + 1) * D], h[p][c][:], start=True, stop=True)
                    if c < N_ACT_CHUNKS:
                        nc.scalar.activation(
                            h[q][c][0:D, :],
                            ps[:],
                            mybir.ActivationFunctionType.Relu,
                        )
                    else:
                        nc.vector.tensor_scalar_max(h[q][c][0:D, :], ps[:], 0.0)

            # Final Linear(100 -> 10), no ReLU. Input parity after 1000 layers
            # is h[0].
            out_sb = mpool.tile([D_OUT, B_CORE], f32, tag="out")
            pf = N_LAYERS % 2
            for c, cw in enumerate(CHUNKS):
                ps = pspool.tile([D_OUT, cw], f32, tag=f"ps{c}", name=f"psf{c}")
                nc.tensor.matmul(ps[:], wf_tile[:], h[pf][c][:], start=True, stop=True)
                nc.scalar.copy(out_sb[:, offs[c] : offs[c + 1]], ps[:])
            nc.sync.dma_start(out_e[:], out_sb[:])

    nc.finalize()
    return nc


def _get_nc(mm_dtype):
    nc = _BUILT.get(mm_dtype)
    if nc is None:
        nc = _build(mm_dtype)
        _BUILT[mm_dtype] = nc
    return nc


def _prep_inputs(x, W, b, Wf, bf):
    x = np.asarray(x, dtype=np.float32)
    W = np.asarray(W, dtype=np.float32)
    b = np.asarray(b, dtype=np.float32)
    Wf = np.asarray(Wf, dtype=np.float32)
    bf = np.asarray(bf, dtype=np.float32)

    # Augmented, transposed weight stack grouped for contiguous DMA:
    # wt[g, p, j*D + m] = Waug[g*W_PER_DMA + j, p, m], Waug[l] = [W[l].T ; b[l]].
    waug = np.concatenate([W.transpose(0, 2, 1), b[:, None, :]], axis=1)
    n_groups = N_LAYERS // W_PER_DMA
    wt = np.ascontiguousarray(
        waug.reshape(n_groups, W_PER_DMA, K, D)
        .transpose(0, 2, 1, 3)
        .reshape(n_groups, K, W_PER_DMA * D)
    )

    xt = np.empty((K, B), dtype=np.float32)
    xt[:D] = x.T
    xt[D] = 1.0

    wft = np.concatenate([Wf.T, bf[None, :]], axis=0)  # [K, D_OUT]
    wft = np.ascontiguousarray(wft, dtype=np.float32)
    return wt, xt, wft


def run(x, W, b, Wf, bf, mm_dtype=None, trace=False):
    from concourse.bass_utils import run_bass_kernel_spmd

    if mm_dtype is None:
        mm_dtype = MM_DTYPE
    nc = _get_nc(mm_dtype)
    wt, xt, wft = _prep_inputs(x, W, b, Wf, bf)
    in_maps = [
        {
            "wt": wt,
            "xt": np.ascontiguousarray(xt[:, i * B_CORE : (i + 1) * B_CORE]),
            "wft": wft,
        }
        for i in range(N_CORES)
    ]
    res = run_bass_kernel_spmd(
        nc, in_maps, core_ids=list(range(N_CORES)), trace=trace
    )
    out = np.concatenate([res.results[i]["out"] for i in range(N_CORES)], axis=1)
    return np.ascontiguousarray(out.T, dtype=np.float32), res


def kernel(x, W, b, Wf, bf):
    out, _ = run(x, W, b, Wf, bf)
    return out


# revision 15
# speedup vs baseline: 1.0214x; 1.0214x over previous
"""Trainium2 Bass kernel for nn_BigNet: 1000x (Linear(100,100)+ReLU) -> Linear(100,10).

Strategy (data-parallel over 8 NeuronCores, batch 8192 -> 1024/core):
  - Activations live in SBUF transposed: h = [feature(+ones row), batch_cols].
  - Each layer: psum = Waug_l.T @ h_aug  (bias folded in as an extra ones-row
    contraction, K=101 <= 128 so it is free on the PE), then ReLU PSUM->SBUF.
  - ReLU work is split across ScalarE (ACT) and VectorE (DVE) by batch-column
    chunks so neither engine is the serial bottleneck.
  - Weights stream from HBM (40MB can't fit in 24MB SBUF), prefetched in
    groups of layers, double-buffered.
  - Final 100->10 layer folded the same way, output [10, 1024] per core,
    gathered and transposed on the host.
"""

import sys

if "/opt/trn_rl_repo" not in sys.path:
    sys.path.insert(0, "/opt/trn_rl_repo")

import numpy as np

N_LAYERS, D, D_OUT, B, N_CORES = 1000, 100, 10, 8192, 8
K = D + 1  # augmented contraction dim (ones row carries the bias)
B_CORE = B // N_CORES  # 1024 batch columns per core

# Batch-column chunks per layer: first group -> ScalarE relu, second -> VectorE.
# Sizes balance measured op costs: ACT=(FD+312)/1.2GHz, DVE=(FD+152)/0.96GHz.
CHUNKS = (230, 230, 282, 282)
N_ACT_CHUNKS = 2  # chunks 0..1 on ScalarE, rest on VectorE
W_PER_DMA = 8  # layers per weight-prefetch DMA

MM_DTYPE = "float32r"  # "float32" (4 cyc/row, exact) or "float32r" (1 cyc/row)

_BUILT = {}


def _build(mm_dtype):
    import concourse.bacc as bacc
    import concourse.mybir as mybir
    from concourse.tile import TileContext

    f32 = mybir.dt.float32
    # float32r: PE streams at 1 cyc/row (vs 4 for f32); producers (DMA/ACT/DVE)
    # must declare f32r output so walrus sees rounded inputs to the matmul.
    DT = mybir.dt.float32r if mm_dtype == "float32r" else f32

    offs = [0]
    for cw in CHUNKS:
        offs.append(offs[-1] + cw)
    assert offs[-1] == B_CORE

    nc = bacc.Bacc(None, target_bir_lowering=False)
    n_groups = N_LAYERS // W_PER_DMA
    # One contiguous [K, W_PER_DMA*D] block per group so each weight DMA is a
    # single linear HBM burst (strided layout measured ~25GB/s; linear is fast).
    wt_e = nc.dram_tensor(
        "wt", [n_groups, K, D * W_PER_DMA], DT, kind="ExternalInput"
    )
    xt_e = nc.dram_tensor("xt", [K, B_CORE], DT, kind="ExternalInput")
    wft_e = nc.dram_tensor("wft", [K, D_OUT], DT, kind="ExternalInput")
    out_e = nc.dram_tensor("out", [D_OUT, B_CORE], f32, kind="ExternalOutput")

    with TileContext(nc) as tc:
        with (
            tc.tile_pool(name="h", bufs=1) as hpool,
            tc.tile_pool(name="w", bufs=4) as wpool,
            tc.tile_pool(name="ps", bufs=1, space="PSUM") as pspool,
            tc.tile_pool(name="misc", bufs=1) as mpool,
        ):
            wf_tile = mpool.tile([K, D_OUT], DT, tag="wf")
            nc.sync.dma_start(wf_tile[:], wft_e[:])

            # Persistent ping-pong activation tiles, one pair per chunk.
            # Row D (index 100) holds the constant 1.0 that multiplies the
            # bias row of the augmented weights.
            h = [
                [
                    hpool.tile([K, CHUNKS[c]], DT, tag=f"h{p}_{c}", name=f"h{p}_{c}")
                    for c in range(len(CHUNKS))
                ]
                for p in range(2)
            ]
            for c in range(len(CHUNKS)):
                nc.sync.dma_start(h[0][c][:], xt_e[:, offs[c] : offs[c + 1]])
                # Ones row for the odd-parity tiles comes from xt's row 100
                # (memset can't encode float32r).
                nc.sync.dma_start(
                    h[1][c][D:K, :], xt_e[D:K, offs[c] : offs[c + 1]]
                )

            wtile = None
            for l in range(N_LAYERS):
                j = l % W_PER_DMA
                if j == 0:
                    g = l // W_PER_DMA
                    wtile = wpool.tile([K, D * W_PER_DMA], DT, tag="w", name="wtile")
                    eng = nc.sync if g % 2 == 0 else nc.gpsimd
                    eng.dma_start(wtile[:], wt_e[g])
                p, q = l % 2, (l + 1) % 2
                for c in (0, 2, 1, 3):
                    cw = CHUNKS[c]
                    ps = pspool.tile([D, cw], f32, tag=f"ps{c}", name=f"ps{c}")
                    nc.tensor.matmul(ps[:], wtile[:, j * D : (j + 1) * D], h[p][c][:], start=True, stop=True)
                    if c < N_ACT_CHUNKS:
                        nc.scalar.activation(
                            h[q][c][0:D, :],
                            ps[:],
                            mybir.ActivationFunctionType.Relu,
                        )
                    else:
                        nc.vector.tensor_scalar_max(h[q][c][0:D, :], ps[:], 0.0)

            # Final Linear(100 -> 10), no ReLU. Input parity after 1000 layers
            # is h[0].
            out_sb = mpool.tile([D_OUT, B_CORE], f32, tag="out")
            pf = N_LAYERS % 2
            for c, cw in enumerate(CHUNKS):
                ps = pspool.tile([D_OUT, cw], f32, tag=f"ps{c}", name=f"psf{c}")
                nc.tensor.matmul(ps[:], wf_tile[:], h[pf][c][:], start=True, stop=True)
                nc.scalar.copy(out_sb[:, offs[c] : offs[c + 1]], ps[:])
            nc.sync.dma_start(out_e[:], out_sb[:])

    nc.finalize()
    return nc


def _get_nc(mm_dtype):
    nc = _BUILT.get(mm_dtype)
    if nc is None:
        nc = _build(mm_dtype)
        _BUILT[mm_dtype] = nc
    return nc


def _prep_inputs(x, W, b, Wf, bf):
    x = np.asarray(x, dtype=np.float32)
    W = np.asarray(W, dtype=np.float32)
    b = np.asarray(b, dtype=np.float32)
    Wf = np.asarray(Wf, dtype=np.float32)
    bf = np.asarray(bf, dtype=np.float32)

    # Augmented, transposed weight stack grouped for contiguous DMA:
    # wt[g, p, j*D + m] = Waug[g*W_PER_DMA + j, p, m], Waug[l] = [W[l].T ; b[l]].
    waug = np.concatenate([W.transpose(0, 2, 1), b[:, None, :]], axis=1)
    n_groups = N_LAYERS // W_PER_DMA
    wt = np.ascontiguousarray(
        waug.reshape(n_groups, W_PER_DMA, K, D)
        .transpose(0, 2, 1, 3)
        .reshape(n_groups, K, W_PER_DMA * D)
    )

    xt = np.empty((K, B), dtype=np.float32)
    xt[:D] = x.T
    xt[D] = 1.0

    wft = np.concatenate([Wf.T, bf[None, :]], axis=0)  # [K, D_OUT]
    wft = np.ascontiguousarray(wft, dtype=np.float32)
    return wt, xt, wft


def run(x, W, b, Wf, bf, mm_dtype=None, trace=False):
    from concourse.bass_utils import run_bass_kernel_spmd

    if mm_dtype is None:
        mm_dtype = MM_DTYPE
    nc = _get_nc(mm_dtype)
    wt, xt, wft = _prep_inputs(x, W, b, Wf, bf)
    in_maps = [
        {
            "wt": wt,
            "xt": np.ascontiguousarray(xt[:, i * B_CORE : (i + 1) * B_CORE]),
            "wft": wft,
        }
        for i in range(N_CORES)
    ]
    res = run_bass_kernel_spmd(
        nc, in_maps, core_ids=list(range(N_CORES)), trace=trace
    )
    out = np.concatenate([res.results[i]["out"] for i in range(N_CORES)], axis=1)
    return np.ascontiguousarray(out.T, dtype=np.float32), res


def kernel(x, W, b, Wf, bf):
    out, _ = run(x, W, b, Wf, bf)
    return out


# revision 17
# speedup vs baseline: 1.0222x; 1.0008x over previous
"""Trainium2 Bass kernel for nn_BigNet: 1000x (Linear(100,100)+ReLU) -> Linear(100,10).

Strategy (data-parallel over 8 NeuronCores, batch 8192 -> 1024/core):
  - Activations live in SBUF transposed: h = [feature(+ones row), batch_cols].
  - Each layer: psum = Waug_l.T @ h_aug  (bias folded in as an extra ones-row
    contraction, K=101 <= 128 so it is free on the PE), then ReLU PSUM->SBUF.
  - ReLU work is split across ScalarE (ACT) and VectorE (DVE) by batch-column
    chunks so neither engine is the serial bottleneck.
  - Weights stream from HBM (40MB can't fit in 24MB SBUF), prefetched in
    groups of layers, double-buffered.
  - Final 100->10 layer folded the same way, output [10, 1024] per core,
    gathered and transposed on the host.
"""

import sys

if "/opt/trn_rl_repo" not in sys.path:
    sys.path.insert(0, "/opt/trn_rl_repo")

import numpy as np

N_LAYERS, D, D_OUT, B, N_CORES = 1000, 100, 10, 8192, 8
K = D + 1  # augmented contraction dim (ones row carries the bias)
B_CORE = B // N_CORES  # 1024 batch columns per core

# Batch-column chunks per layer: first group -> ScalarE relu, second -> VectorE.
# Sizes balance measured op costs: ACT=(FD+312)/1.2GHz, DVE=(FD+152)/0.96GHz.
CHUNKS = (230, 230, 282, 282)
N_ACT_CHUNKS = 2  # chunks 0..1 on ScalarE, rest on VectorE
W_PER_DMA = 8  # layers per weight-prefetch DMA

MM_DTYPE = "float32r"  # "float32" (4 cyc/row, exact) or "float32r" (1 cyc/row)

_BUILT = {}


def _build(mm_dtype):
    import concourse.bacc as bacc
    import concourse.mybir as mybir
    from concourse.tile import TileContext

    f32 = mybir.dt.float32
    # float32r: PE streams in a single fp32 HIGH pass (vs 2 for f32); producers
    # (DMA/ACT/DVE) must declare f32r output so walrus sees rounded matmul inputs.
    # (bf16 stationary + f32r moving was tried and is rejected by walrus:
    # "Mixing of 32-bit and non-32-bit Matmult inputs not supported".)
    DT = f32 if mm_dtype == "float32" else mybir.dt.float32r
    DT_W = DT

    offs = [0]
    for cw in CHUNKS:
        offs.append(offs[-1] + cw)
    assert offs[-1] == B_CORE

    nc = bacc.Bacc(None, target_bir_lowering=False)
    n_groups = N_LAYERS // W_PER_DMA
    # One contiguous [K, W_PER_DMA*D] block per group so each weight DMA is a
    # single linear HBM burst (strided layout measured ~25GB/s; linear is fast).
    wt_e = nc.dram_tensor(
        "wt", [n_groups, K, D * W_PER_DMA], DT_W, kind="ExternalInput"
    )
    xt_e = nc.dram_tensor("xt", [K, B_CORE], DT, kind="ExternalInput")
    wft_e = nc.dram_tensor("wft", [K, D_OUT], DT_W, kind="ExternalInput")
    out_e = nc.dram_tensor("out", [D_OUT, B_CORE], f32, kind="ExternalOutput")

    with TileContext(nc) as tc:
        with (
            tc.tile_pool(name="h", bufs=1) as hpool,
            tc.tile_pool(name="w", bufs=4) as wpool,
            tc.tile_pool(name="ps", bufs=1, space="PSUM") as pspool,
            tc.tile_pool(name="misc", bufs=1) as mpool,
        ):
            wf_tile = mpool.tile([K, D_OUT], DT_W, tag="wf")
            nc.sync.dma_start(wf_tile[:], wft_e[:])

            # Persistent ping-pong activation tiles, one pair per chunk.
            # Row D (index 100) holds the constant 1.0 that multiplies the
            # bias row of the augmented weights.
            h = [
                [
                    hpool.tile([K, CHUNKS[c]], DT, tag=f"h{p}_{c}", name=f"h{p}_{c}")
                    for c in range(len(CHUNKS))
                ]
                for p in range(2)
            ]
            for c in range(len(CHUNKS)):
                nc.sync.dma_start(h[0][c][:], xt_e[:, offs[c] : offs[c + 1]])
                # Ones row for the odd-parity tiles comes from xt's row 100
                # (memset can't encode float32r).
                nc.sync.dma_start(
                    h[1][c][D:K, :], xt_e[D:K, offs[c] : offs[c + 1]]
                )

            wtile = None
            for l in range(N_LAYERS):
                j = l % W_PER_DMA
                if j == 0:
                    g = l // W_PER_DMA
                    wtile = wpool.tile([K, D * W_PER_DMA], DT_W, tag="w", name="wtile")
                    eng = nc.sync if g % 2 == 0 else nc.gpsimd
                    eng.dma_start(wtile[:], wt_e[g])
                p, q = l % 2, (l + 1) % 2
                for c, cw in enumerate(CHUNKS):
                    ps = pspool.tile([D, cw], f32, tag=f"ps{c}", name=f"ps{c}")
                    nc.tensor.matmul(ps[:], wtile[:, j * D : (j + 1) * D], h[p][c][:], start=True, stop=True)
                    if c < N_ACT_CHUNKS:
                        nc.scalar.activation(
                            h[q][c][0:D, :],
                            ps[:],
                            mybir.ActivationFunctionType.Relu,
                        )
                    else:
                        nc.vector.tensor_scalar_max(h[q][c][0:D, :], ps[:], 0.0)

            # Final Linear(100 -> 10), no ReLU. Input parity after 1000 layers
            # is h[0].
            out_sb = mpool.tile([D_OUT, B_CORE], f32, tag="out")
            pf = N_LAYERS % 2
            for c, cw in enumerate(CHUNKS):
                ps = pspool.tile([D_OUT, cw], f32, tag=f"ps{c}", name=f"psf{c}")
                nc.tensor.matmul(ps[:], wf_tile[:], h[pf][c][:], start=True, stop=True)
                nc.scalar.copy(out_sb[:, offs[c] : offs[c + 1]], ps[:])
            nc.sync.dma_start(out_e[:], out_sb[:])

    nc.finalize()
    return nc


def _get_nc(mm_dtype):
    nc = _BUILT.get(mm_dtype)
    if nc is None:
        nc = _build(mm_dtype)
        _BUILT[mm_dtype] = nc
    return nc


def _prep_inputs(x, W, b, Wf, bf, mm_dtype):
    x = np.asarray(x, dtype=np.float32)
    W = np.asarray(W, dtype=np.float32)
    b = np.asarray(b, dtype=np.float32)
    Wf = np.asarray(Wf, dtype=np.float32)
    bf = np.asarray(bf, dtype=np.float32)

    # Augmented, transposed weight stack grouped for contiguous DMA:
    # wt[g, p, j*D + m] = Waug[g*W_PER_DMA + j, p, m], Waug[l] = [W[l].T ; b[l]].
    waug = np.concatenate([W.transpose(0, 2, 1), b[:, None, :]], axis=1)
    n_groups = N_LAYERS // W_PER_DMA
    wt = np.ascontiguousarray(
        waug.reshape(n_groups, W_PER_DMA, K, D)
        .transpose(0, 2, 1, 3)
        .reshape(n_groups, K, W_PER_DMA * D)
    )

    xt = np.empty((K, B), dtype=np.float32)
    xt[:D] = x.T
    xt[D] = 1.0

    wft = np.concatenate([Wf.T, bf[None, :]], axis=0)  # [K, D_OUT]
    wft = np.ascontiguousarray(wft, dtype=np.float32)
    return wt, xt, wft


def run(x, W, b, Wf, bf, mm_dtype=None, trace=False):
    from concourse.bass_utils import run_bass_kernel_spmd

    if mm_dtype is None:
        mm_dtype = MM_DTYPE
    nc = _get_nc(mm_dtype)
    wt, xt, wft = _prep_inputs(x, W, b, Wf, bf, mm_dtype)
    in_maps = [
        {
            "wt": wt,
            "xt": np.ascontiguousarray(xt[:, i * B_CORE : (i + 1) * B_CORE]),
            "wft": wft,
        }
        for i in range(N_CORES)
    ]
    res = run_bass_kernel_spmd(
        nc, in_maps, core_ids=list(range(N_CORES)), trace=trace
    )
    out = np.concatenate([res.results[i]["out"] for i in range(N_CORES)], axis=1)
    return np.ascontiguousarray(out.T, dtype=np.float32), res


def kernel(x, W, b, Wf, bf):
    out, _ = run(x, W, b, Wf, bf)
    return out


# revision 18
# speedup vs baseline: 1.0354x; 1.0129x over previous
"""Trainium2 Bass kernel for nn_BigNet: 1000x (Linear(100,100)+ReLU) -> Linear(100,10).

Strategy (data-parallel over 8 NeuronCores, batch 8192 -> 1024/core):
  - Activations live in SBUF transposed: h = [feature(+ones row), batch_cols].
  - Each layer: psum = Waug_l.T @ h_aug  (bias folded in as an extra ones-row
    contraction, K=101 <= 128 so it is free on the PE), then ReLU PSUM->SBUF.
  - ReLU work is split across ScalarE (ACT) and VectorE (DVE) by batch-column
    chunks so neither engine is the serial bottleneck.
  - Weights stream from HBM (40MB can't fit in 24MB SBUF), prefetched in
    groups of layers, double-buffered.
  - Final 100->10 layer folded the same way, output [10, 1024] per core,
    gathered and transposed on the host.
"""

import sys

if "/opt/trn_rl_repo" not in sys.path:
    sys.path.insert(0, "/opt/trn_rl_repo")

import numpy as np

N_LAYERS, D, D_OUT, B, N_CORES = 1000, 100, 10, 8192, 8
K = D + 1  # augmented contraction dim (ones row carries the bias)
B_CORE = B // N_CORES  # 1024 batch columns per core

# Batch-column chunks per layer: first group -> ScalarE relu, second -> VectorE.
# Sizes balance measured op costs: ACT=(FD+312)/1.2GHz, DVE=(FD+152)/0.96GHz.
CHUNKS = (230, 230, 282, 282)
N_ACT_CHUNKS = 2  # chunks 0..1 on ScalarE, rest on VectorE
W_PER_DMA = 8  # layers per weight-prefetch DMA

MM_DTYPE = "float32r"  # "float32" (4 cyc/row, exact) or "float32r" (1 cyc/row)

_BUILT = {}


def _build(mm_dtype):
    import concourse.bacc as bacc
    import concourse.mybir as mybir
    from concourse.tile import TileContext

    f32 = mybir.dt.float32
    # float32r: PE streams in a single fp32 HIGH pass (vs 2 for f32); producers
    # (DMA/ACT/DVE) must declare f32r output so walrus sees rounded matmul inputs.
    # (bf16 stationary + f32r moving was tried and is rejected by walrus:
    # "Mixing of 32-bit and non-32-bit Matmult inputs not supported".)
    DT = f32 if mm_dtype == "float32" else mybir.dt.float32r
    DT_W = DT

    offs = [0]
    for cw in CHUNKS:
        offs.append(offs[-1] + cw)
    assert offs[-1] == B_CORE

    nc = bacc.Bacc(None, target_bir_lowering=False)
    n_groups = N_LAYERS // W_PER_DMA
    # One contiguous [K, W_PER_DMA*D] block per group so each weight DMA is a
    # single linear HBM burst (strided layout measured ~25GB/s; linear is fast).
    wt_e = nc.dram_tensor(
        "wt", [n_groups, K, D * W_PER_DMA], DT_W, kind="ExternalInput"
    )
    # xt is packed per chunk: chunk c occupies the contiguous block
    # [K*offs[c], K*offs[c+1]) laid out as [K, cw] row-major, so the initial
    # activation DMAs are linear HBM bursts (strided reads run ~25GB/s).
    xt_e = nc.dram_tensor("xt", [K * B_CORE], DT, kind="ExternalInput")
    wft_e = nc.dram_tensor("wft", [K, D_OUT], DT_W, kind="ExternalInput")
    out_e = nc.dram_tensor("out", [D_OUT, B_CORE], f32, kind="ExternalOutput")

    with TileContext(nc) as tc:
        with (
            tc.tile_pool(name="h", bufs=1) as hpool,
            tc.tile_pool(name="w", bufs=4) as wpool,
            tc.tile_pool(name="ps", bufs=1, space="PSUM") as pspool,
            tc.tile_pool(name="misc", bufs=1) as mpool,
        ):
            wf_tile = mpool.tile([K, D_OUT], DT_W, tag="wf")
            nc.gpsimd.dma_start(wf_tile[:], wft_e[:])

            # Persistent ping-pong activation tiles, one pair per chunk.
            # Row D (index 100) holds the constant 1.0 that multiplies the
            # bias row of the augmented weights.
            h = [
                [
                    hpool.tile([K, CHUNKS[c]], DT, tag=f"h{p}_{c}", name=f"h{p}_{c}")
                    for c in range(len(CHUNKS))
                ]
                for p in range(2)
            ]
            # Initial loads go on the gpsimd queue so the first weight group
            # (sync queue) streams in parallel — the old serial order cost
            # ~50us of head latency before the first matmul.
            for c, cw in enumerate(CHUNKS):
                blk = xt_e[K * offs[c] : K * offs[c + 1]].rearrange(
                    "(k w) -> k w", w=cw
                )
                nc.gpsimd.dma_start(h[0][c][:], blk)
                # Ones row for the odd-parity tiles comes from the block's
                # row 100 (memset can't encode float32r).
                nc.gpsimd.dma_start(h[1][c][D:K, :], blk[D:K, :])

            wtile = None
            for l in range(N_LAYERS):
                j = l % W_PER_DMA
                if j == 0:
                    g = l // W_PER_DMA
                    wtile = wpool.tile([K, D * W_PER_DMA], DT_W, tag="w", name="wtile")
                    eng = nc.sync if g % 2 == 0 else nc.gpsimd
                    eng.dma_start(wtile[:], wt_e[g])
                p, q = l % 2, (l + 1) % 2
                for c, cw in enumerate(CHUNKS):
                    ps = pspool.tile([D, cw], f32, tag=f"ps{c}", name=f"ps{c}")
                    nc.tensor.matmul(ps[:], wtile[:, j * D : (j + 1) * D], h[p][c][:], start=True, stop=True)
                    if c < N_ACT_CHUNKS:
                        nc.scalar.activation(
                            h[q][c][0:D, :],
                            ps[:],
                            mybir.ActivationFunctionType.Relu,
                        )
                    else:
                        nc.vector.tensor_scalar_max(h[q][c][0:D, :], ps[:], 0.0)

            # Final Linear(100 -> 10), no ReLU. Input parity after 1000 layers
            # is h[0].
            out_sb = mpool.tile([D_OUT, B_CORE], f32, tag="out")
            pf = N_LAYERS % 2
            for c, cw in enumerate(CHUNKS):
                ps = pspool.tile([D_OUT, cw], f32, tag=f"ps{c}", name=f"psf{c}")
                nc.tensor.matmul(ps[:], wf_tile[:], h[pf][c][:], start=True, stop=True)
                nc.scalar.copy(out_sb[:, offs[c] : offs[c + 1]], ps[:])
            nc.sync.dma_start(out_e[:], out_sb[:])

    nc.finalize()
    return nc


def _get_nc(mm_dtype):
    nc = _BUILT.get(mm_dtype)
    if nc is None:
        nc = _build(mm_dtype)
        _BUILT[mm_dtype] = nc
    return nc


def _prep_inputs(x, W, b, Wf, bf, mm_dtype):
    x = np.asarray(x, dtype=np.float32)
    W = np.asarray(W, dtype=np.float32)
    b = np.asarray(b, dtype=np.float32)
    Wf = np.asarray(Wf, dtype=np.float32)
    bf = np.asarray(bf, dtype=np.float32)

    # Augmented, transposed weight stack grouped for contiguous DMA:
    # wt[g, p, j*D + m] = Waug[g*W_PER_DMA + j, p, m], Waug[l] = [W[l].T ; b[l]].
    waug = np.concatenate([W.transpose(0, 2, 1), b[:, None, :]], axis=1)
    n_groups = N_LAYERS // W_PER_DMA
    wt = np.ascontiguousarray(
        waug.reshape(n_groups, W_PER_DMA, K, D)
        .transpose(0, 2, 1, 3)
        .reshape(n_groups, K, W_PER_DMA * D)
    )

    xt = np.empty((K, B), dtype=np.float32)
    xt[:D] = x.T
    xt[D] = 1.0
    # Pack per-core, per-chunk contiguous blocks: [sum_c K*cw] per core.
    offs = [0]
    for cw in CHUNKS:
        offs.append(offs[-1] + cw)
    xt_packed = np.empty((N_CORES, K * B_CORE), dtype=np.float32)
    for i in range(N_CORES):
        col0 = i * B_CORE
        for c, cw in enumerate(CHUNKS):
            xt_packed[i, K * offs[c] : K * offs[c + 1]] = xt[
                :, col0 + offs[c] : col0 + offs[c + 1]
            ].ravel()

    wft = np.concatenate([Wf.T, bf[None, :]], axis=0)  # [K, D_OUT]
    wft = np.ascontiguousarray(wft, dtype=np.float32)
    return wt, xt_packed, wft


def run(x, W, b, Wf, bf, mm_dtype=None, trace=False):
    from concourse.bass_utils import run_bass_kernel_spmd

    if mm_dtype is None:
        mm_dtype = MM_DTYPE
    nc = _get_nc(mm_dtype)
    wt, xt_packed, wft = _prep_inputs(x, W, b, Wf, bf, mm_dtype)
    in_maps = [
        {"wt": wt, "xt": xt_packed[i], "wft": wft} for i in range(N_CORES)
    ]
    res = run_bass_kernel_spmd(
        nc, in_maps, core_ids=list(range(N_CORES)), trace=trace
    )
    out = np.concatenate([res.results[i]["out"] for i in range(N_CORES)], axis=1)
    return np.ascontiguousarray(out.T, dtype=np.float32), res


def kernel(x, W, b, Wf, bf):
    out, _ = run(x, W, b, Wf, bf)
    return out


# revision 19
# speedup vs baseline: 1.0452x; 1.0095x over previous
"""Trainium2 Bass kernel for nn_BigNet: 1000x (Linear(100,100)+ReLU) -> Linear(100,10).

Strategy (data-parallel over 8 NeuronCores, batch 8192 -> 1024/core):
  - Activations live in SBUF transposed: h = [feature(+ones row), batch_cols].
  - Each layer: psum = Waug_l.T @ h_aug  (bias folded in as an extra ones-row
    contraction, K=101 <= 128 so it is free on the PE), then ReLU PSUM->SBUF.
  - ReLU work is split across ScalarE (ACT) and VectorE (DVE) by batch-column
    chunks so neither engine is the serial bottleneck.
  - Weights stream from HBM (40MB can't fit in 24MB SBUF), prefetched in
    groups of layers, double-buffered.
  - Final 100->10 layer folded the same way, output [10, 1024] per core,
    gathered and transposed on the host.
"""

import sys

if "/opt/trn_rl_repo" not in sys.path:
    sys.path.insert(0, "/opt/trn_rl_repo")

import numpy as np

N_LAYERS, D, D_OUT, B, N_CORES = 1000, 100, 10, 8192, 8
K = D + 1  # augmented contraction dim (ones row carries the bias)
B_CORE = B // N_CORES  # 1024 batch columns per core

# Batch-column chunks per layer: first group -> ScalarE relu, second -> VectorE.
# Sizes balance measured op costs: ACT=(FD+312)/1.2GHz, DVE=(FD+152)/0.96GHz.
CHUNKS = (230, 230, 282, 282)
N_ACT_CHUNKS = 2  # chunks 0..1 on ScalarE, rest on VectorE
W_PER_DMA = 8  # layers per weight-prefetch DMA

MM_DTYPE = "float32r"  # "float32" (4 cyc/row, exact) or "float32r" (1 cyc/row)

_BUILT = {}


def _build(mm_dtype):
    import concourse.bacc as bacc
    import concourse.mybir as mybir
    from concourse.tile import TileContext

    f32 = mybir.dt.float32
    # float32r: PE streams in a single fp32 HIGH pass (vs 2 for f32); producers
    # (DMA/ACT/DVE) must declare f32r output so walrus sees rounded matmul inputs.
    # (bf16 stationary + f32r moving was tried and is rejected by walrus:
    # "Mixing of 32-bit and non-32-bit Matmult inputs not supported".)
    DT = f32 if mm_dtype == "float32" else mybir.dt.float32r
    DT_W = DT

    offs = [0]
    for cw in CHUNKS:
        offs.append(offs[-1] + cw)
    assert offs[-1] == B_CORE

    nc = bacc.Bacc(None, target_bir_lowering=False)
    n_groups = N_LAYERS // W_PER_DMA
    # One contiguous [K, W_PER_DMA*D] block per group so each weight DMA is a
    # single linear HBM burst (strided layout measured ~25GB/s; linear is fast).
    wt_e = nc.dram_tensor(
        "wt", [n_groups, K, D * W_PER_DMA], DT_W, kind="ExternalInput"
    )
    # xt is packed per chunk: chunk c occupies the contiguous block
    # [K*offs[c], K*offs[c+1]) laid out as [K, cw] row-major, so the initial
    # activation DMAs are linear HBM bursts (strided reads run ~25GB/s).
    xt_e = nc.dram_tensor("xt", [K * B_CORE], DT, kind="ExternalInput")
    wft_e = nc.dram_tensor("wft", [K, D_OUT], DT_W, kind="ExternalInput")
    out_e = nc.dram_tensor("out", [D_OUT, B_CORE], f32, kind="ExternalOutput")

    with TileContext(nc) as tc:
        with (
            tc.tile_pool(name="h", bufs=1) as hpool,
            tc.tile_pool(name="w", bufs=4) as wpool,
            tc.tile_pool(name="ps", bufs=1, space="PSUM") as pspool,
            tc.tile_pool(name="misc", bufs=1) as mpool,
        ):
            # Prefetch the first two weight groups before any activation
            # loads so layer 0's LDWEIGHTS is never gated behind them.
            wtiles0 = []
            for g0 in range(2):
                wt_t = wpool.tile(
                    [K, D * W_PER_DMA], DT_W, tag="w", name="wtile"
                )
                eng = nc.sync if g0 % 2 == 0 else nc.gpsimd
                eng.dma_start(wt_t[:], wt_e[g0])
                wtiles0.append(wt_t)

            wf_tile = mpool.tile([K, D_OUT], DT_W, tag="wf")
            nc.gpsimd.dma_start(wf_tile[:], wft_e[:])

            # Persistent ping-pong activation tiles, one pair per chunk.
            # Row D (index 100) holds the constant 1.0 that multiplies the
            # bias row of the augmented weights.
            h = [
                [
                    hpool.tile([K, CHUNKS[c]], DT, tag=f"h{p}_{c}", name=f"h{p}_{c}")
                    for c in range(len(CHUNKS))
                ]
                for p in range(2)
            ]
            # Initial loads go on the gpsimd queue so the first weight group
            # (sync queue) streams in parallel — the old serial order cost
            # ~50us of head latency before the first matmul.
            for c, cw in enumerate(CHUNKS):
                blk = xt_e[K * offs[c] : K * offs[c + 1]].rearrange(
                    "(k w) -> k w", w=cw
                )
                nc.gpsimd.dma_start(h[0][c][:], blk)
                # Ones row for the odd-parity tiles comes from the block's
                # row 100 (memset can't encode float32r).
                nc.gpsimd.dma_start(h[1][c][D:K, :], blk[D:K, :])

            wtile = None
            for l in range(N_LAYERS):
                j = l % W_PER_DMA
                if j == 0:
                    g = l // W_PER_DMA
                    if g < 2:
                        wtile = wtiles0[g]
                    else:
                        wtile = wpool.tile(
                            [K, D * W_PER_DMA], DT_W, tag="w", name="wtile"
                        )
                        eng = nc.sync if g % 2 == 0 else nc.gpsimd
                        eng.dma_start(wtile[:], wt_e[g])
                p, q = l % 2, (l + 1) % 2
                for c, cw in enumerate(CHUNKS):
                    ps = pspool.tile([D, cw], f32, tag=f"ps{c}", name=f"ps{c}")
                    nc.tensor.matmul(ps[:], wtile[:, j * D : (j + 1) * D], h[p][c][:], start=True, stop=True)
                    if c < N_ACT_CHUNKS:
                        nc.scalar.activation(
                            h[q][c][0:D, :],
                            ps[:],
                            mybir.ActivationFunctionType.Relu,
                        )
                    else:
                        nc.vector.tensor_scalar_max(h[q][c][0:D, :], ps[:], 0.0)

            # Final Linear(100 -> 10), no ReLU. Input parity after 1000 layers
            # is h[0].
            out_sb = mpool.tile([D_OUT, B_CORE], f32, tag="out")
            pf = N_LAYERS % 2
            for c, cw in enumerate(CHUNKS):
                ps = pspool.tile([D_OUT, cw], f32, tag=f"ps{c}", name=f"psf{c}")
                nc.tensor.matmul(ps[:], wf_tile[:], h[pf][c][:], start=True, stop=True)
                nc.scalar.copy(out_sb[:, offs[c] : offs[c + 1]], ps[:])
            nc.sync.dma_start(out_e[:], out_sb[:])

    nc.finalize()
    return nc


def _get_nc(mm_dtype):
    nc = _BUILT.get(mm_dtype)
    if nc is None:
        nc = _build(mm_dtype)
        _BUILT[mm_dtype] = nc
    return nc


def _prep_inputs(x, W, b, Wf, bf, mm_dtype):
    x = np.asarray(x, dtype=np.float32)
    W = np.asarray(W, dtype=np.float32)
    b = np.asarray(b, dtype=np.float32)
    Wf = np.asarray(Wf, dtype=np.float32)
    bf = np.asarray(bf, dtype=np.float32)

    # Augmented, transposed weight stack grouped for contiguous DMA:
    # wt[g, p, j*D + m] = Waug[g*W_PER_DMA + j, p, m], Waug[l] = [W[l].T ; b[l]].
    waug = np.concatenate([W.transpose(0, 2, 1), b[:, None, :]], axis=1)
    n_groups = N_LAYERS // W_PER_DMA
    wt = np.ascontiguousarray(
        waug.reshape(n_groups, W_PER_DMA, K, D)
        .transpose(0, 2, 1, 3)
        .reshape(n_groups, K, W_PER_DMA * D)
    )

    xt = np.empty((K, B), dtype=np.float32)
    xt[:D] = x.T
    xt[D] = 1.0
    # Pack per-core, per-chunk contiguous blocks: [sum_c K*cw] per core.
    offs = [0]
    for cw in CHUNKS:
        offs.append(offs[-1] + cw)
    xt_packed = np.empty((N_CORES, K * B_CORE), dtype=np.float32)
    for i in range(N_CORES):
        col0 = i * B_CORE
        for c, cw in enumerate(CHUNKS):
            xt_packed[i, K * offs[c] : K * offs[c + 1]] = xt[
                :, col0 + offs[c] : col0 + offs[c + 1]
            ].ravel()

    wft = np.concatenate([Wf.T, bf[None, :]], axis=0)  # [K, D_OUT]
    wft = np.ascontiguousarray(wft, dtype=np.float32)
    return wt, xt_packed, wft


def run(x, W, b, Wf, bf, mm_dtype=None, trace=False):
    from concourse.bass_utils import run_bass_kernel_spmd

    if mm_dtype is None:
        mm_dtype = MM_DTYPE
    nc = _get_nc(mm_dtype)
    wt, xt_packed, wft = _prep_inputs(x, W, b, Wf, bf, mm_dtype)
    in_maps = [
        {"wt": wt, "xt": xt_packed[i], "wft": wft} for i in range(N_CORES)
    ]
    res = run_bass_kernel_spmd(
        nc, in_maps, core_ids=list(range(N_CORES)), trace=trace
    )
    out = np.concatenate([res.results[i]["out"] for i in range(N_CORES)], axis=1)
    return np.ascontiguousarray(out.T, dtype=np.float32), res


def kernel(x, W, b, Wf, bf):
    out, _ = run(x, W, b, Wf, bf)
    return out
